# revision 36
# baseline (speedup 1.0000x reference)
"""MDTA (Restormer channel-attention) on 8 TRN2 NeuronCores — v2.

Sharding: (batch=2) x (4 row-blocks of 48 image rows) -> 8 cores.

Per-core pipeline:
  - q,k path in fp8: 1x1 conv as DoubleRow matmuls (paired 96-channel
    contraction), depthwise 3x3 as fp8 diag matmuls with vertical tap
    pairs DoubleRow'd (4.5 effective passes instead of 9), outputs
    DMA-XBAR-transposed into [pixel, channel] tiles, per-head Gram
    accumulation on the PE. fp8 error washes out: the l2 norms come from
    the Gram diagonal of the same quantized tensors, and the logits are
    tiny cosines.
  - v path in bf16: 1x1 conv on the PE, depthwise 3x3 on the DVE as
    scalar_tensor_tensor FMAs (4x mode).
  - Grams AllReduced across each batch's 4 cores, then a small softmax
    chain; attn and proj fold into M = Wproj @ blockdiag(A_h^T) so the
    output epilogue is a single matmul stream M^T.T @ v with the proj
    PSUM DMA'd straight to DRAM.
"""
import numpy as np
import ml_dtypes
from contextlib import ExitStack

import concourse.bass as bass
import concourse.tile as tile
import concourse.bacc as bacc
import concourse.mybir as mybir
from concourse import bass_utils

BF16 = mybir.dt.bfloat16
F32 = mybir.dt.float32
FP8 = mybir.dt.float8e4
bf = ml_dtypes.bfloat16
f8 = ml_dtypes.float8_e4m3fn
DR = mybir.MatmulPerfMode.DoubleRow
AF = mybir.ActivationFunctionType
ALU = mybir.AluOpType

N_CORES = 8
C = 192
HEADS, HD = 4, 48
IMG = 192
RB = 48                 # image rows per core
PIX = RB * IMG          # 9216 valid px
SR = RB + 2             # 50 slab rows
SW8 = 208               # fp8 slab row width (16B aligned)
SWV = 194               # bf16 v slab row width
NT = SR * IMG // 384    # 25 conv tiles over the slab
NG = RB // 4            # 12 groups of 4 output rows (768 px)
NB = PIX // 128         # 72 gram pixel blocks
NT2 = PIX // 512        # 18 proj tiles
DXS = (-1, 0, 1)
TAPS = [(dy, dx) for dy in (-1, 0, 1) for dx in (-1, 0, 1)]
EPS_NORM = 1e-12
EPS_TEMP = 1e-06

_cache = {}


def build_nc(reps: int = 1, single: bool = False, v_mode: str = 'dve'):
    nc = bacc.Bacc("TRN2", target_bir_lowering=False, debug=False,
                   num_devices=1 if single else N_CORES)
    t = {}
    t['xb'] = nc.dram_tensor("xb", [C, SR * IMG], BF16, kind="ExternalInput")
    t['x8'] = nc.dram_tensor("x8", [96, 2 * SR * IMG], FP8, kind="ExternalInput")
    t['wq8'] = nc.dram_tensor("wq8", [96, 2 * 384], FP8, kind="ExternalInput")
    t['wv0'] = nc.dram_tensor("wv0", [128, 224], BF16, kind="ExternalInput")
    t['wv1'] = nc.dram_tensor("wv1", [64, 224], BF16, kind="ExternalInput")
    t['dgp'] = nc.dram_tensor("dgp", [128, 3 * 768], FP8, kind="ExternalInput")
    t['dgs'] = nc.dram_tensor("dgs", [128, 3 * 384], FP8, kind="ExternalInput")
    t['wdv'] = nc.dram_tensor("wdv", [112, 18], F32, kind="ExternalInput")
    t['dgv'] = nc.dram_tensor("dgv", [112, 2 * 9 * 112], BF16, kind="ExternalInput")
    t['wp48'] = nc.dram_tensor("wp48", [48, 4 * 192], BF16, kind="ExternalInput")
    t['sel4'] = nc.dram_tensor("sel4", [4, 4 * 48], F32, kind="ExternalInput")
    t['maskG'] = nc.dram_tensor("maskG", [48, 576], F32, kind="ExternalInput")
    t['id48'] = nc.dram_tensor("id48", [48, 48], F32, kind="ExternalInput")
    t['tempq'] = nc.dram_tensor("tempq", [48, 4], F32, kind="ExternalInput")
    t['out'] = nc.dram_tensor("out", [C, PIX], F32, kind="ExternalOutput")

    with tile.TileContext(nc) as tc:
        for _ in range(reps):
            _one_rep(tc, t, single)
    nc.compile()
    return nc


def _one_rep(tc, t, single):
    nc = tc.nc
    with ExitStack() as octx:
        P = octx.enter_context(tc.tile_pool(name="persist", bufs=1))
        dram = octx.enter_context(tc.tile_pool(name="dram", bufs=2, space="DRAM"))

        # ---- persistent tiles ----------------------------------------
        s8 = [P.tile([128, SR * SW8], FP8, tag=f"s8_{i}", name=f"s8_{i}")
              for i in range(3)]
        vs = [P.tile([112, SR * SWV], BF16, tag=f"vs{i}", name=f"vs{i}")
              for i in range(2)]
        va = P.tile([112, PIX], BF16, tag="va")
        vb = P.tile([112, PIX], BF16, tag="vb")
        wq8 = P.tile([96, 2 * 384], FP8, tag="wq8")
        wv0 = P.tile([128, 224], BF16, tag="wv0")
        wv1 = P.tile([64, 224], BF16, tag="wv1")
        dgp = P.tile([128, 3 * 768], FP8, tag="dgp")
        dgs = P.tile([128, 3 * 384], FP8, tag="dgs")
        wdv = P.tile([112, 18], F32, tag="wdv")
        dgv = P.tile([112, 2 * 9 * 112], BF16, tag="dgv")
        wp48 = P.tile([48, 4 * 192], BF16, tag="wp48")
        sel4 = P.tile([4, 4 * 48], F32, tag="sel4")
        maskG = P.tile([48, 576], F32, tag="maskG")
        id48 = P.tile([48, 48], F32, tag="id48")
        tempq = P.tile([48, 4], F32, tag="tempq")
        gsb = P.tile([48, 576], F32, tag="gsb")
        G = P.tile([48, 576], F32, tag="G")
        MTa = P.tile([112, 192], BF16, tag="MTa")
        MTb = P.tile([112, 192], BF16, tag="MTb")

        # critical-path first on sync; consts on gpsimd swdge
        nc.sync.dma_start(wq8[:], t['wq8'].ap())
        nc.sync.dma_start(wv0[:], t['wv0'].ap())
        nc.sync.dma_start(wv1[:], t['wv1'].ap())
        def load_late_weights():
            nc.sync.dma_start(wdv[:], t['wdv'].ap())
            nc.sync.dma_start(dgp[:], t['dgp'].ap())
            nc.sync.dma_start(dgs[:], t['dgs'].ap())
            nc.sync.dma_start(dgv[:], t['dgv'].ap())
            nc.scalar.dma_start(wp48[:], t['wp48'].ap())
            nc.scalar.dma_start(sel4[:], t['sel4'].ap())
            nc.scalar.dma_start(maskG[:], t['maskG'].ap())
            nc.scalar.dma_start(id48[:], t['id48'].ap())
        nc.sync.dma_start(tempq[:], t['tempq'].ap())

        # pre-warm ACT tables (sqrt + exp sets) so the softmax chain
        # doesn't pay table loads on the critical path
        warm = P.tile([1, 4], F32, tag="warm")
        nc.scalar.sqrt(warm[:], tempq[0:1, :])
        nc.scalar.activation(warm[:], tempq[0:1, :], AF.Exp)

        # zero pad columns (col 0 and 193 are read by dx shifts)
        for s in s8:
            v3 = s[:, :].rearrange("p (r c) -> p r c", c=SW8)
            nc.vector.memset(v3[:, :, 0:1], 0)
            nc.vector.memset(v3[:, :, 193:194], 0)
        for s in vs:
            v3 = s[:, :].rearrange("p (r c) -> p r c", c=SWV)
            nc.vector.memset(v3[:, :, 0:1], 0)
            nc.vector.memset(v3[:, :, 193:194], 0)

        cp = [0]

        def ccopy(dst, src):
            # PSUM->SBUF copies: GPSIMD cannot touch PSUM, so rotate
            # 2x ACT : 1x DVE
            k = cp[0] % 3
            cp[0] += 1
            if k == 2:
                nc.vector.tensor_copy(dst, src)
            else:
                nc.scalar.copy(dst, src)

        # ---- main phase: conv, v-dw, qk-dw, transpose, gram ----------
        with ExitStack() as ctx:
            xp = ctx.enter_context(tc.tile_pool(name="xp", bufs=2))
            cps = ctx.enter_context(tc.tile_pool(name="cps", bufs=2, space="PSUM"))
            wpp = ctx.enter_context(tc.tile_pool(name="wpp", bufs=4, space="PSUM"))
            gp = ctx.enter_context(tc.tile_pool(name="gp", bufs=1, space="PSUM"))
            dtp = ctx.enter_context(tc.tile_pool(name="dtp", bufs=3))
            qkp = ctx.enter_context(tc.tile_pool(name="qkp", bufs=3))

            cpn = [0]

            def conv_ps():
                cpn[0] += 1
                pool = cps if cpn[0] % 3 == 0 else wpp
                tag = "cps" if pool is cps else "dwps"
                return pool.tile([128, 384], F32, tag=tag, name="convps")

            wq83 = wq8[:, :].rearrange("p (two n) -> p two n", two=2)
            s83 = [s[:, :].rearrange("p (r c) -> p r c", c=SW8) for s in s8]
            vs3 = [s[:, :].rearrange("p (r c) -> p r c", c=SWV) for s in vs]
            x8d = t['x8'].ap().rearrange("p (two n) -> p two n", two=2)

            # qk depthwise (fp8 DR) + dma transpose + gram, emitted
            # interleaved with the conv windows below
            g1 = gp.tile([48, 384], F32, tag="g1")   # Gq_h: [q.q | q.k] x4
            g2 = gp.tile([48, 192], F32, tag="g2")   # Gk_h: k.k x4
            dgp3 = dgp[:, :].rearrange("p (c x two m) -> p c x two m",
                                       c=3, x=3, two=2)
            dgs3 = dgs[:, :].rearrange("p (c x m) -> p c x m", c=3, x=3)

            def _gram(gg, qk_t):
                for b in range(6):
                    first = (gg == 0 and b == 0)
                    last = (gg == NG - 1 and b == 5)
                    qb = qk_t[:, b * 384:(b + 1) * 384]
                    qb2 = qb.rearrange("p (two c) -> p two c", two=2)
                    for h in range(HEADS):
                        nc.tensor.matmul(
                            g1[:, h * 96:(h + 1) * 96],
                            qb[:, h * 48:(h + 1) * 48],
                            qb2[:, :, h * 48:(h + 1) * 48],
                            start=first, stop=last)
                        nc.tensor.matmul(
                            g2[:, h * 48:(h + 1) * 48],
                            qb[:, 192 + h * 48:192 + (h + 1) * 48],
                            qb[:, 192 + h * 48:192 + (h + 1) * 48],
                            start=first, stop=last)

            pend = [None]

            def emit_dw_group(g):
                qkt = qkp.tile([128, 6 * 384], BF16, tag="qkt")
                qv = qkt[:, :].rearrange("p (b c) -> p b c", c=384)
                for ck in range(3):
                    dt = dtp.tile([128, 768], BF16, tag="dt")
                    for half in range(2):
                        tn = 2 * g + half      # 2-row dw tile index
                        ps = wpp.tile([128, 384], F32, tag="dwps")
                        for rr in range(2):
                            r = 2 * tn + rr    # output row 0..47
                            reg = ps[:, rr * 192:(rr + 1) * 192]
                            for dxi in range(3):
                                pair = s83[ck][:, r:r + 3:2,
                                               1 + DXS[dxi]:193 + DXS[dxi]]
                                nc.tensor.matmul(
                                    reg, dgp3[:, ck, dxi, :, :], pair,
                                    start=(dxi == 0), stop=False,
                                    perf_mode=DR)
                            for dxi in range(3):
                                sing = s83[ck][:, r + 1,
                                               1 + DXS[dxi]:193 + DXS[dxi]]
                                nc.tensor.matmul(
                                    reg, dgs3[:, ck, dxi, :], sing,
                                    start=False, stop=(dxi == 2))
                        if cp[0] % 2:
                            nc.scalar.copy(dt[:, half * 384:(half + 1) * 384],
                                           ps[:])
                        else:
                            nc.vector.tensor_copy(
                                dt[:, half * 384:(half + 1) * 384], ps[:])
                        cp[0] += 1
                    nc.sync.dma_start_transpose(
                        qv[:, :, ck * 128:(ck + 1) * 128], dt[:])
                if pend[0] is not None:
                    _gram(pend[0][0], pend[0][1])
                pend[0] = (g, qkt)

            # conv with windowed double-buffered x loads (5 tiles/window),
            # qk-dw groups interleaved as their slab rows become available
            WNT = 5
            next_g = 0
            for w in range((NT + WNT - 1) // WNT):
                n0 = w * WNT
                nw = min(WNT, NT - n0)
                ws = slice(n0 * 384, (n0 + nw) * 384)
                x0 = xp.tile([128, WNT * 384], BF16, tag="x0")
                x1 = xp.tile([64, WNT * 384], BF16, tag="x1")
                x8t = xp.tile([96, 2 * WNT * 384], FP8, tag="x8")
                x83 = x8t[:, :].rearrange("p (two n) -> p two n", two=2)
                nc.sync.dma_start(x0[:, 0:nw * 384], t['xb'].ap()[0:128, ws])
                nc.sync.dma_start(x1[:, 0:nw * 384], t['xb'].ap()[128:192, ws])
                nc.sync.dma_start(x83[:, :, 0:nw * 384], x8d[:, :, ws])
                for k in range(nw):
                    n = n0 + k
                    sl = slice(k * 384, (k + 1) * 384)
                    # v chunks first (they gate the long DVE v-dw chain)
                    for i in range(2):
                        ps = conv_ps()
                        nc.tensor.matmul(ps[0:112, :],
                                         wv0[:, i * 112:(i + 1) * 112],
                                         x0[:, sl], start=True, stop=False)
                        nc.tensor.matmul(ps[0:112, :],
                                         wv1[:, i * 112:(i + 1) * 112],
                                         x1[:, sl], start=False, stop=True)
                        ccopy(vs3[i][:, 2 * n:2 * n + 2, 1:193], ps[0:112, :])
                    for ck in range(3):
                        ps = conv_ps()
                        nc.tensor.matmul(
                            ps[:], wq83[:, :, ck * 128:(ck + 1) * 128],
                            x83[:, :, sl],
                            start=True, stop=True, perf_mode=DR)
                        ccopy(s83[ck][:, 2 * n:2 * n + 2, 1:193], ps[:])
                if w == 0:
                    load_late_weights()
                gmax = -1   # no interleave: copy engines saturate during conv
                if w == (NT + WNT - 1) // WNT - 1:
                    gmax = NG - 2
                while next_g <= gmax:
                    emit_dw_group(next_g)
                    next_g += 1

            # v depthwise: DVE rows [0, RPE0) as mult(4x)+add(2x) chains in
            # quarter slices; PE rows [RPE0, 48) as bf16 diag matmuls.
            RPE0 = 40                   # first PE-handled v row
            vtp = ctx.enter_context(tc.tile_pool(name="vtp", bufs=2))
            QR = RPE0 // 4  # 11-row quarter chains
            for i, dst in ((0, va), (1, vb)):
                d3 = dst[:, :].rearrange("p (r c) -> p r c", c=IMG)
                for qq in range(4):
                    r0 = qq * QR
                    od = d3[:, r0:r0 + QR, :]
                    for tt, (dy, dx) in enumerate(TAPS):
                        win = vs3[i][:, 1 + dy + r0:1 + dy + r0 + QR,
                                     1 + dx:1 + dx + IMG]
                        wcol = wdv[0:112, 9 * i + tt:9 * i + tt + 1]
                        eng = nc.gpsimd if tt == 8 else nc.vector
                        if tt == 0:
                            nc.vector.tensor_scalar_mul(od, win, wcol)
                        else:
                            vt = vtp.tile([112, QR * IMG], BF16, tag="vt")
                            t3 = vt[:, :].rearrange("p (r c) -> p r c", c=IMG)
                            eng.tensor_scalar_mul(t3[:, :, :], win, wcol)
                            hs = slice(r0 * IMG, (r0 + QR) * IMG)
                            eng.tensor_add(dst[:, hs], dst[:, hs],
                                           vt[:, :])
            dgv3 = dgv[:, :].rearrange("p (i t m) -> p i t m", i=2, t=9)  # m=112
            for i, dst in ((0, va), (1, vb)):
                d3 = dst[:, :].rearrange("p (r c) -> p r c", c=IMG)
                for tn in range((RB - RPE0) // 2):
                    ps = wpp.tile([128, 384], F32, tag="dwps")
                    for rr in range(2):
                        r = RPE0 + 2 * tn + rr
                        for tt, (dy, dx) in enumerate(TAPS):
                            nc.tensor.matmul(
                                ps[0:112, rr * 192:(rr + 1) * 192],
                                dgv3[:, i, tt, :],
                                vs3[i][:, 1 + dy + r, 1 + dx:1 + dx + IMG],
                                start=(tt == 0), stop=(tt == 8))
                    ccopy(d3[:, RPE0 + 2 * tn:RPE0 + 2 * tn + 2, :],
                          ps[0:112, :])

            nc.scalar.activation(warm[:], tempq[0:1, :], AF.Exp)
            nc.scalar.sqrt(warm[:], tempq[0:1, :])
            while next_g < NG:
                emit_dw_group(next_g)
                next_g += 1
            _gram(pend[0][0], pend[0][1])
            nc.scalar.copy(gsb[:, 0:384], g1[:])
            nc.vector.tensor_copy(gsb[:, 384:576], g2[:])

        # ---- AllReduce Grams within each batch's 4 cores -------------
        if single:
            nc.vector.tensor_copy(G[:], gsb[:])
        else:
            arin = dram.tile([48, 576], F32, tag="arin")
            arout = dram.tile([48, 576], F32, tag="arout")
            nc.sync.dma_start(arin[:], gsb[:])
            nc.gpsimd.collective_compute(
                "AllReduce", ALU.add,
                replica_groups=[[0, 1, 2, 3], [4, 5, 6, 7]],
                ins=[arin.opt()], outs=[arout.opt()])
            nc.sync.dma_start(G[:], arout[:])

        # ---- softmax + M = Wp @ blockdiag(A^T) -----------------------
        with ExitStack() as ctx:
            sp = ctx.enter_context(tc.tile_pool(name="sp", bufs=1))
            p2 = ctx.enter_context(tc.tile_pool(name="p2", bufs=1, space="PSUM"))

            gm = sp.tile([48, 576], F32, tag="gm")
            nc.vector.tensor_mul(gm[:], G[:], maskG[:])
            s = sp.tile([48, 8], F32, tag="s")
            gmq = gm[:, 0:384].rearrange("p (h c) -> p h c", h=4)
            gmk = gm[:, 384:576].rearrange("p (h c) -> p h c", h=4)
            nc.vector.tensor_reduce(s[:, 0:4], gmq[:, :, :],
                                    axis=mybir.AxisListType.X, op=ALU.add)
            nc.vector.tensor_reduce(s[:, 4:8], gmk[:, :, :],
                                    axis=mybir.AxisListType.X, op=ALU.add)
            nrm = sp.tile([48, 8], F32, tag="nrm")
            nc.scalar.sqrt(nrm[:], s[:])
            nc.vector.tensor_scalar_max(nrm[:], nrm[:], EPS_NORM)
            r = sp.tile([48, 8], F32, tag="r")
            nc.vector.reciprocal(r[:], nrm[:])
            rqt = sp.tile([48, 4], F32, tag="rqt")
            nc.vector.tensor_mul(rqt[:], r[:, 0:4], tempq[:])

            rkp = p2.tile([4, 48], F32, tag="rkp")
            nc.tensor.transpose(rkp[:], r[:, 4:8], id48[:])
            rks = sp.tile([4, 48], F32, tag="rks")
            nc.vector.tensor_copy(rks[:], rkp[:])
            rkbp = p2.tile([48, 192], F32, tag="rkbp")
            for h in range(HEADS):
                nc.tensor.matmul(rkbp[:, h * 48:(h + 1) * 48],
                                 sel4[:, h * 48:(h + 1) * 48], rks[:],
                                 start=True, stop=True)
            rkb = sp.tile([48, 192], F32, tag="rkb")
            nc.vector.tensor_copy(rkb[:], rkbp[:])

            L = sp.tile([48, 192], F32, tag="L")
            for h in range(HEADS):
                nc.vector.tensor_scalar_mul(
                    L[:, h * 48:(h + 1) * 48],
                    G[:, h * 96 + 48:(h + 1) * 96],
                    rqt[:, h:h + 1])
            nc.vector.tensor_mul(L[:], L[:], rkb[:])
            E = sp.tile([48, 192], F32, tag="E")
            nc.scalar.activation(E[:], L[:], AF.Exp)
            den = sp.tile([48, 4], F32, tag="den")
            E3 = E[:, :].rearrange("p (h c) -> p h c", h=4)
            nc.vector.tensor_reduce(den[:], E3[:, :, :],
                                    axis=mybir.AxisListType.X, op=ALU.add)
            rd = sp.tile([48, 4], F32, tag="rd")
            nc.vector.reciprocal(rd[:], den[:])
            A = sp.tile([48, 192], BF16, tag="A")
            for h in range(HEADS):
                nc.vector.tensor_scalar_mul(A[:, h * 48:(h + 1) * 48],
                                            E[:, h * 48:(h + 1) * 48],
                                            rd[:, h:h + 1])
            # MT_h[e, o] = sum_d A_h[d, e] WpT[h*48+d, o]; odd heads land
            # at base partition 64 so MTa/MTb assemble without shifts
            mt01 = p2.tile([112, 192], F32, tag="mt01")
            mt23 = p2.tile([112, 192], F32, tag="mt23")
            for h in range(HEADS):
                dst = (mt01 if h < 2 else mt23)
                p0 = (h % 2) * 64
                nc.tensor.matmul(dst[p0:p0 + 48, :],
                                 A[:, h * 48:(h + 1) * 48],
                                 wp48[:, h * 192:(h + 1) * 192],
                                 start=True, stop=True)
            nc.vector.memset(MTa[32:64, :], 0)
            nc.vector.memset(MTb[32:64, :], 0)
            nc.scalar.copy(MTa[0:48, :], mt01[0:48, :])
            nc.vector.tensor_copy(MTa[64:112, :], mt01[64:112, :])
            nc.scalar.copy(MTb[0:48, :], mt23[0:48, :])
            nc.vector.tensor_copy(MTb[64:112, :], mt23[64:112, :])

        # ---- proj epilogue: out = (MT).T @ v, PSUM -> DRAM -----------
        with ExitStack() as ctx:
            p3 = ctx.enter_context(tc.tile_pool(name="p3", bufs=3, space="PSUM"))
            op = ctx.enter_context(tc.tile_pool(name="op", bufs=3))
            oeng = [nc.scalar, nc.vector]
            for n in range(NT2):
                sl = slice(n * 512, (n + 1) * 512)
                po0 = p3.tile([128, 512], F32, tag="po0")
                po1 = p3.tile([64, 512], F32, tag="po1")
                nc.tensor.matmul(po0[:], MTa[:, 0:128], va[:, sl],
                                 start=True, stop=False)
                nc.tensor.matmul(po0[:], MTb[:, 0:128], vb[:, sl],
                                 start=False, stop=True)
                nc.tensor.matmul(po1[:], MTa[:, 128:192], va[:, sl],
                                 start=True, stop=False)
                nc.tensor.matmul(po1[:], MTb[:, 128:192], vb[:, sl],
                                 start=False, stop=True)
                if n % 2 == 0:
                    ot0 = op.tile([128, 1024], F32, tag="ot0")
                    ot1 = op.tile([64, 1024], F32, tag="ot1")
                half = (n % 2) * 512
                e = oeng[n % 2]
                e2 = oeng[(n + 1) % 2]
                (e.copy if e is nc.scalar else e.tensor_copy)(
                    ot0[:, half:half + 512], po0[:])
                (e2.copy if e2 is nc.scalar else e2.tensor_copy)(
                    ot1[:, half:half + 512], po1[:])
                if n % 2 == 1:
                    sl2 = slice((n - 1) * 512, (n + 1) * 512)
                    nc.sync.dma_start(t['out'].ap()[0:128, sl2], ot0[:])
                    nc.sync.dma_start(t['out'].ap()[128:192, sl2], ot1[:])


# ---------------------------------------------------------------------
# host side
# ---------------------------------------------------------------------

def prep_inputs(x, w_qkv, w_dw, w_proj, log_temperature):
    x = np.asarray(x, np.float32)
    w_qkv = np.asarray(w_qkv, np.float32)
    w_dw = np.asarray(w_dw, np.float32).reshape(3 * C, 3, 3)
    w_proj = np.asarray(w_proj, np.float32)
    lt = np.asarray(log_temperature, np.float32).reshape(HEADS)

    # qk 1x1 conv weights, fp8 DoubleRow pairing over 96-channel halves
    Wqk = w_qkv[0:384, :]                       # [384 out, 192 in]
    arr = np.ascontiguousarray(Wqk.T)           # [192 in, 384 out]
    wq8 = np.concatenate([arr[0:96][:, None, :], arr[96:192][:, None, :]],
                         axis=1).reshape(96, 768).astype(f8)
    Wv = w_qkv[384:576, :]
    wvT = np.ascontiguousarray(Wv.T)            # [192 in, 192 out]
    wvp = np.zeros((192, 224), np.float32)      # 112-wide chunks, hole 48:64
    for i in range(2):
        wvp[:, i * 112:i * 112 + 48] = wvT[:, i * 96:i * 96 + 48]
        wvp[:, i * 112 + 64:i * 112 + 112] = wvT[:, i * 96 + 48:i * 96 + 96]
    wv0 = wvp[0:128].astype(bf)
    wv1 = wvp[128:192].astype(bf)

    # depthwise diag weights
    wq3 = w_dw[0:384]                           # [384, 3, 3]
    dgp = np.zeros((128, 3, 3, 2, 128), np.float32)
    dgs = np.zeros((128, 3, 3, 128), np.float32)
    idx = np.arange(128)
    for ck in range(3):
        ch = wq3[ck * 128:(ck + 1) * 128]       # [128, 3, 3]
        for dxi in range(3):
            dgp[idx, ck, dxi, 0, idx] = ch[:, 0, dxi]
            dgp[idx, ck, dxi, 1, idx] = ch[:, 2, dxi]
            dgs[idx, ck, dxi, idx] = ch[:, 1, dxi]
    dgp = dgp.reshape(128, 3 * 768).astype(f8)
    dgs = dgs.reshape(128, 3 * 384).astype(f8)

    wdv = np.zeros((112, 18), np.float32)
    dgv = np.zeros((112, 2, 9, 112), np.float32)
    hole = np.r_[0:48, 64:112]                  # 96 live rows of 112
    for i in range(2):
        ch = w_dw[384 + i * 96:384 + (i + 1) * 96]
        for tt, (dy, dx) in enumerate(TAPS):
            wdv[hole, 9 * i + tt] = ch[:, dy + 1, dx + 1]
            dgv[hole, i, tt, hole] = ch[:, dy + 1, dx + 1]
    dgv = dgv.reshape(112, 2 * 9 * 112).astype(bf)

    wp48 = np.zeros((48, 4 * 192), np.float32)
    for h in range(HEADS):
        wp48[:, h * 192:(h + 1) * 192] = w_proj[:, h * 48:(h + 1) * 48].T
    wp48 = wp48.astype(bf)

    sel4 = np.zeros((4, 4 * 48), np.float32)
    for h in range(HEADS):
        sel4[h, h * 48:(h + 1) * 48] = 1.0
    maskG = np.zeros((48, 576), np.float32)
    d = np.arange(48)
    for h in range(HEADS):
        maskG[d, h * 96 + d] = 1.0              # diag(q.q)
        maskG[d, 384 + h * 48 + d] = 1.0        # diag(k.k)
    id48 = np.eye(48, dtype=np.float32)
    temp = np.log1p(np.exp(lt)) + EPS_TEMP
    tempq = np.tile(temp[None, :], (48, 1)).astype(np.float32)

    in_maps = []
    for core in range(N_CORES):
        b, rb = core // 4, core % 4
        r0 = rb * RB
        slab = np.zeros((C, SR, IMG), np.float32)
        lo, hi = r0 - 1, r0 + RB + 1
        slo, shi = max(lo, 0), min(hi, IMG)
        slab[:, slo - lo:shi - lo, :] = x[b, :, slo:shi, :]
        xs = slab.reshape(C, SR * IMG)
        x8 = np.concatenate([xs[0:96][:, None, :], xs[96:192][:, None, :]],
                            axis=1).reshape(96, 2 * SR * IMG).astype(f8)
        in_maps.append({
            "xb": np.ascontiguousarray(xs).astype(bf), "x8": x8,
            "wq8": wq8, "wv0": wv0, "wv1": wv1, "dgp": dgp, "dgs": dgs,
            "wdv": wdv, "dgv": dgv, "wp48": wp48, "sel4": sel4, "maskG": maskG,
            "id48": id48, "tempq": tempq,
        })
    return in_maps


def assemble(results):
    out = np.zeros((2, C, IMG, IMG), np.float32)
    for core in range(N_CORES):
        b, rb = core // 4, core % 4
        out[b, :, rb * RB:(rb + 1) * RB, :] = \
            results[core]["out"].reshape(C, RB, IMG)
    return out


def kernel(**inputs) -> np.ndarray:
    if "nc" not in _cache:
        _cache["nc"] = build_nc(reps=1)
    nc = _cache["nc"]
    in_maps = prep_inputs(**inputs)
    res = bass_utils.run_bass_kernel_spmd(
        nc, in_maps, core_ids=list(range(N_CORES)))
    return assemble(res.results)
